# revision 12
# baseline (speedup 1.0000x reference)
"""Trainium2 Bass kernel for Swin-style window attention.

Shapes (hardcoded): x (64, 256, 25, 25) f32, w_qkv (256, 768), w_out (256, 256),
bias_table (2401, 8), rel_pos_indices (625, 625) int32.

Sharding: data-parallel over the window/batch dim b: 8 windows per core on 8
NeuronCores. Weights/bias table replicated.

Per-core dataflow (all on-device):
  - gather bias_full[j, i, h] = bias_table[rel_pos_indices[i, j], h] once via
    indirect DMA (transposing rel_pos_indices via PE transpose first), repack
    to planar per-head tiles biasT[jt][:, h*625:(h+1)*625].
  - per window:
      qT/kT (head-major, d on partitions) = w_qkv.T @ x      (w_qkv stationary)
      v (n on partitions, d free)          = x.T @ w_qkv_v    (x stationary)
      per head h, j-tile jt:
        psum  = bias (via identity-matmul accumulate) + kT_h.T @ qT_h  (= simT[j, i])
        E = exp(psum)                                  (ScalarE, PSUM -> SBUF)
      per head: outT_u (33, 625) = [v_h | 1].T @ E     (row 32 = softmax denom)
      normalize by reciprocal of denom, final = w_out.T @ outT -> y (d-major)
"""

import os
from contextlib import ExitStack

import numpy as np

import concourse.bass as bass
from concourse import bacc
import concourse.mybir as mybir
import concourse.tile as tile
from concourse.masks import make_identity

F32 = mybir.dt.float32
I32 = mybir.dt.int32

B, D, HWD, N = 64, 256, 25, 625
HEADS, DH = 8, 32
NCORES = 8
BW = B // NCORES          # windows per core
NT = 5                    # n-dim tiles
TP = 125                  # partitions per n-tile
TABLE = 2401
SCALE = DH ** -0.5
CH = [(0, 512), (512, 113)]   # free-dim chunks of 625 for fp32 matmul (N<=512)


def build_bias_sbuf(ctx, tc, nc, table_d, rel_d, bias_pool, ident):
    """Gather biasT[jt] (125, 8*625) planar per-head tiles: biasT[jt][j, h*625+i]
    = bias_table[rel[i, j], h]. Returns list of 5 tiles."""
    biasT = [
        bias_pool.tile([TP, HEADS * N], F32, tag=f"biasT{jt}", name=f"biasT{jt}")
        for jt in range(NT)
    ]
    with tc.tile_pool(name="bias_build", bufs=1) as bp, \
         tc.tile_pool(name="bias_build_ps", bufs=2, space="PSUM") as bpp:
        # straight (i-part, j-free) int32 tiles then cast to f32
        relf = []
        for it in range(NT):
            r_i = bp.tile([TP, N], I32, tag=f"rel_i{it}", name=f"rel_i{it}")
            nc.sync.dma_start(out=r_i[:], in_=rel_d[it * TP:(it + 1) * TP, :])
            r_f = bp.tile([TP, N], F32, tag=f"rel_f{it}", name=f"rel_f{it}")
            nc.vector.tensor_copy(out=r_f[:], in_=r_i[:])
            relf.append(r_f)
        for jt in range(NT):
            idxT = bp.tile([TP, N], I32, tag="idxT")
            for it in range(NT):
                tps = bpp.tile([TP, TP], F32, tag="tr", space="PSUM")
                nc.tensor.transpose(
                    out=tps[:],
                    in_=relf[it][:, jt * TP:(jt + 1) * TP],
                    identity=ident[:TP, :TP],
                )
                nc.vector.tensor_copy(
                    out=idxT[:, it * TP:(it + 1) * TP], in_=tps[:]
                )
            # gather rows of (2401, 8) table: out interleaved (i outer, h inner).
            # HW indirect DMA honors at most 25 offsets per partition per
            # descriptor (measured), so chunk the 625 offsets.
            GC = 25
            gi = bp.tile([TP, N * HEADS], F32, tag="gather")
            for i0 in range(0, N, GC):
                nc.gpsimd.indirect_dma_start(
                    out=gi[:, i0 * HEADS:(i0 + GC) * HEADS],
                    out_offset=None,
                    in_=table_d[:],
                    in_offset=bass.IndirectOffsetOnAxis(
                        ap=idxT[:, i0:i0 + GC], axis=0),
                )
            gi3 = gi[:].rearrange("p (i h) -> p h i", h=HEADS)
            for h in range(HEADS):
                nc.vector.tensor_copy(
                    out=biasT[jt][:, h * N:(h + 1) * N], in_=gi3[:, h, :]
                )
    return biasT


def window_body(ctx, tc, nc, w, x_d, y_d, wq, wo, biasT, pools):
    """Emit one window's attention."""
    sp, mp, ep, wp = pools  # sim psum, misc psum, E sbuf, work sbuf pools

    # ---- load x (256, 625) as 2 tiles ----
    xw = []
    for c in range(2):
        xt = wp.tile([128, N], F32, tag=f"x{c}", bufs=2, name=f"x{c}_{w}")
        nc.sync.dma_start(out=xt[:], in_=x_d[w, c * 128:(c + 1) * 128, :])
        xw.append(xt)

    # ---- qT/kT: (512, 625) head-major = w_qkv[:, :512].T @ x ----
    qk = []
    for m in range(4):
        ps = mp.tile([128, N], F32, tag="mpsum", space="PSUM", name=f"qk_ps{m}_{w}")
        for (o, sz) in CH:
            for c in range(2):
                nc.tensor.matmul(
                    out=ps[:, o:o + sz],
                    lhsT=wq[c][:, m * 128:(m + 1) * 128],
                    rhs=xw[c][:, o:o + sz],
                    start=(c == 0), stop=(c == 1),
                )
        t = wp.tile([128, N], F32, tag=f"qk{m}", bufs=1, name=f"qk{m}_{w}")
        if m < 2:  # q tiles: fold in softmax scale
            nc.vector.tensor_scalar_mul(t[:], ps[:], SCALE)
        else:
            nc.vector.tensor_copy(out=t[:], in_=ps[:])
        qk.append(t)

    # ---- v: (625, 256) n-major, padded per head with ones col: (125, 8*33) ----
    vaug = []
    for nt in range(NT):
        ps = mp.tile([TP, 256], F32, tag="mpsum", space="PSUM", name=f"v_ps{nt}_{w}")
        for c in range(2):
            nc.tensor.matmul(
                out=ps[:],
                lhsT=xw[c][:, nt * TP:(nt + 1) * TP],
                rhs=wq[c][:, 512:768],
                start=(c == 0), stop=(c == 1),
            )
        va = wp.tile([TP, HEADS * 33], F32, tag=f"va{nt}", bufs=1, name=f"va{nt}_{w}")
        va3 = va[:].rearrange("p (h e) -> p h e", h=HEADS)
        nc.vector.memset(va3[:, :, 32], 1.0)
        nc.vector.tensor_copy(
            out=va3[:, :, 0:32],
            in_=ps[:].rearrange("p (h e) -> p h e", h=HEADS),
        )
        vaug.append(va)

    # ---- per head: sim + bias -> exp -> attn@v ----
    colsum = wp.tile([HEADS, N], F32, tag="colsum", bufs=1, name=f"colsum_{w}")
    out_u = [
        wp.tile([128, N], F32, tag=f"outu{t}", bufs=1, name=f"outu{t}_{w}")
        for t in range(2)
    ]
    for h in range(HEADS):
        g, m = h // 4, h % 4
        et = []
        for jt in range(NT):
            ps = sp.tile([TP, N], F32, tag="spsum", space="PSUM",
                         name=f"sim_ps{h}_{jt}_{w}")
            for (o, sz) in CH:
                # bias preload via identity matmul (sets has_written)
                nc.tensor.matmul(
                    out=ps[:, o:o + sz],
                    lhsT=ident_g(nc)[:TP, :TP],
                    rhs=biasT[jt][:, h * N + o:h * N + o + sz],
                    start=True, stop=False,
                )
                # simT[j, i] accumulate: kT_h stationary, qT_h moving
                nc.tensor.matmul(
                    out=ps[:, o:o + sz],
                    lhsT=qk[2 + g][32 * m:32 * m + 32, jt * TP:(jt + 1) * TP],
                    rhs=qk[g][32 * m:32 * m + 32, o:o + sz],
                    start=False, stop=True,
                    tile_position=(32 * m, 0),
                )
            e = ep.tile([TP, N], F32, tag="E", bufs=10, name=f"E{h}_{jt}_{w}")
            nc.scalar.activation(e[:], ps[:], mybir.ActivationFunctionType.Exp)
            et.append(e)
        # attn@v (transposed out): [v_h | 1].T @ E -> (33, 625)
        po = mp.tile([33, N], F32, tag="mpsum", space="PSUM", name=f"av_ps{h}_{w}")
        for jt in range(NT):
            for (o, sz) in CH:
                nc.tensor.matmul(
                    out=po[:, o:o + sz],
                    lhsT=vaug[jt][:, h * 33:(h + 1) * 33],
                    rhs=et[jt][:, o:o + sz],
                    start=(jt == 0), stop=(jt == NT - 1),
                )
        nc.vector.tensor_copy(
            out=out_u[g][32 * m:32 * m + 32, :], in_=po[0:32, :]
        )
        cstage = wp.tile([1, N], F32, tag="cstage", bufs=2, name=f"cstage{h}_{w}")
        nc.vector.tensor_copy(out=cstage[:], in_=po[32:33, :])
        nc.sync.dma_start(out=colsum[h:h + 1, :], in_=cstage[:])

    # ---- normalize: out_n = out_u * (1/colsum) broadcast over dh ----
    recip = wp.tile([HEADS, N], F32, tag="recip", bufs=1, name=f"recip_{w}")
    nc.vector.reciprocal(recip[:], colsum[:])
    out_n = [
        wp.tile([128, N], F32, tag=f"outn{t}", bufs=1, name=f"outn{t}_{w}")
        for t in range(2)
    ]
    for t in range(2):
        rps = mp.tile([128, N], F32, tag="mpsum", space="PSUM", name=f"r_ps{t}_{w}")
        for (o, sz) in CH:
            nc.tensor.matmul(
                out=rps[:, o:o + sz],
                lhsT=ind8_g()[:, t * 128:(t + 1) * 128],
                rhs=recip[:, o:o + sz],
                start=True, stop=True,
            )
        nc.vector.tensor_mul(out_n[t][:], out_u[t][:], rps[:])

    # ---- final: y[w] = (w_out.T @ out_n) (256, 625) d-major ----
    for t in range(2):
        ps = mp.tile([128, N], F32, tag="mpsum", space="PSUM", name=f"f_ps{t}_{w}")
        for (o, sz) in CH:
            for c in range(2):
                nc.tensor.matmul(
                    out=ps[:, o:o + sz],
                    lhsT=wo[c][:, t * 128:(t + 1) * 128],
                    rhs=out_n[c][:, o:o + sz],
                    start=(c == 0), stop=(c == 1),
                )
        fin = wp.tile([128, N], F32, tag=f"fin{t}", bufs=1, name=f"fin{t}_{w}")
        nc.vector.tensor_copy(out=fin[:], in_=ps[:])
        nc.sync.dma_start(out=y_d[w, t * 128:(t + 1) * 128, :], in_=fin[:])


_IDENT = None
_IND8 = None


def ident_g(nc):
    return _IDENT


def ind8_g():
    return _IND8


def build_nc(bw=BW):
    nc = bacc.Bacc(target_bir_lowering=False, debug=False)
    x_d = nc.dram_tensor("x", [bw, D, N], F32, kind="ExternalInput")
    wq_d = nc.dram_tensor("w_qkv", [D, 3 * D], F32, kind="ExternalInput")
    wo_d = nc.dram_tensor("w_out", [D, D], F32, kind="ExternalInput")
    tab_d = nc.dram_tensor("bias_table", [TABLE, HEADS], F32, kind="ExternalInput")
    rel_d = nc.dram_tensor("rel_idx", [N, N], I32, kind="ExternalInput")
    y_d = nc.dram_tensor("y", [bw, D, N], F32, kind="ExternalOutput")

    global _IDENT
    with ExitStack() as ctx:
        tc = ctx.enter_context(tile.TileContext(nc))
        const = ctx.enter_context(tc.tile_pool(name="const", bufs=1))
        bias_pool = ctx.enter_context(tc.tile_pool(name="bias", bufs=1))
        ident_gp = const.tile([128, 128], F32, tag="identgp", name="ident_gp")
        make_identity(nc, ident_gp[:])
        _IDENT = const.tile([128, 128], F32, tag="identm", name="identm")
        nc.vector.tensor_copy(out=_IDENT[:], in_=ident_gp[:])
        global _IND8
        _IND8 = const.tile([HEADS, D], F32, tag="ind8", name="ind8")
        nc.gpsimd.memset(_IND8[:], 1.0)
        # keep 1.0 only where 0 <= y - 32p <= 31  (expr OP 0 ? keep : fill)
        nc.gpsimd.affine_select(
            out=_IND8[:], in_=_IND8[:], pattern=[[1, D]], channel_multiplier=-32,
            base=0, compare_op=mybir.AluOpType.is_ge, fill=0.0)
        nc.gpsimd.affine_select(
            out=_IND8[:], in_=_IND8[:], pattern=[[-1, D]], channel_multiplier=32,
            base=31, compare_op=mybir.AluOpType.is_ge, fill=0.0)
        ind8_dve = const.tile([HEADS, D], F32, tag="ind8d", name="ind8_dve")
        nc.vector.tensor_copy(out=ind8_dve[:], in_=_IND8[:])
        _IND8 = ind8_dve
        wq = []
        for c in range(2):
            t = const.tile([128, 3 * D], F32, tag=f"wq{c}", name=f"wq{c}")
            nc.sync.dma_start(out=t[:], in_=wq_d[c * 128:(c + 1) * 128, :])
            wq.append(t)
        wo = []
        for c in range(2):
            t = const.tile([128, D], F32, tag=f"wo{c}", name=f"wo{c}")
            nc.sync.dma_start(out=t[:], in_=wo_d[c * 128:(c + 1) * 128, :])
            wo.append(t)

        biasT = build_bias_sbuf(ctx, tc, nc, tab_d, rel_d, bias_pool, _IDENT)

        # work pools opened after the bias-build transients are released
        sp = ctx.enter_context(tc.tile_pool(name="simps", bufs=2, space="PSUM"))
        mp = ctx.enter_context(tc.tile_pool(name="miscps", bufs=2, space="PSUM"))
        ep = ctx.enter_context(tc.tile_pool(name="esb", bufs=10))
        wp = ctx.enter_context(tc.tile_pool(name="work", bufs=2))

        for w in range(bw):
            window_body(ctx, tc, nc, w, x_d, y_d, wq, wo, biasT, (sp, mp, ep, wp))
    nc.compile()
    return nc


_NC_CACHE = {}


def get_nc(bw=BW):
    if bw not in _NC_CACHE:
        _NC_CACHE[bw] = build_nc(bw)
    return _NC_CACHE[bw]


def make_in_maps(x, w_qkv, w_out, bias_table, rel_pos_indices, ncores=NCORES):
    xs = np.ascontiguousarray(
        np.asarray(x, dtype=np.float32).reshape(B, D, N)
    ).reshape(ncores, B // ncores, D, N)
    wq = np.ascontiguousarray(np.asarray(w_qkv, dtype=np.float32))
    wo = np.ascontiguousarray(np.asarray(w_out, dtype=np.float32))
    tb = np.ascontiguousarray(np.asarray(bias_table, dtype=np.float32))
    ri = np.ascontiguousarray(np.asarray(rel_pos_indices, dtype=np.int32))
    return [
        {"x": np.ascontiguousarray(xs[c]), "w_qkv": wq, "w_out": wo,
         "bias_table": tb, "rel_idx": ri}
        for c in range(ncores)
    ]


def kernel(x, w_qkv, w_out, bias_table, rel_pos_indices):
    from concourse.bass_utils import run_bass_kernel_spmd

    nc = get_nc(BW)
    in_maps = make_in_maps(x, w_qkv, w_out, bias_table, rel_pos_indices)
    res = run_bass_kernel_spmd(nc, in_maps, core_ids=list(range(NCORES)))
    y = np.concatenate([r["y"] for r in res.results], axis=0)  # (64, 256, 625)
    return y.reshape(B, D, HWD, HWD).astype(np.float32)
